# revision 10
# baseline (speedup 1.0000x reference)
"""Trainium2 Bass kernel for nn_GCN_12575664243073 (11-layer GCN + mean-pool + MLP).

Strategy (8 NeuronCores, SPMD):
  - Nodes sharded contiguously by dst across 8 cores (12500 each); small weights
    replicated; full node-feature array re-replicated each layer via AllGather.
  - Per layer: neighbor features fetched with dma_gather (int16 indices, 4
    32768-row banks), aggregated per 256-node window with one-hot matmuls on
    the PE into PSUM (exact segment-sum), then W-matmul + bias + leaky-relu
    with all per-node norms folded into matmul operands.
  - Graph mean-pool folded into layer 11 as per-chunk one-hot matmuls.
  - Tiny 64-row MLP tail (with jax dropout masks) computed on host.
"""
import os
import sys
sys.path.insert(0, "/opt/trn_rl_repo")
import numpy as np
import ml_dtypes

last_exec_time_ns = None

import concourse.bass as bass
import concourse.mybir as mybir
import concourse.bacc as bacc
import concourse.tile as tile
from concourse import bass_utils

BF = ml_dtypes.bfloat16
P = 128
NCORES = 8
SLOPE = 0.01

# problem constants (hardcoded; kernel.py must be self-contained)
N_FULL = 100000
E_FULL = 450000
B_FULL = 64
DIMS_FULL = [128, 80, 160, 112, 160, 176, 96, 144, 96, 128, 96, 160]


def _cdiv(a, b):
    return -(-a // b)


def _pd(d):
    # bf16 gather rows must be a multiple of 128 elems (256B)
    return _cdiv(d, 128) * 128


class Plan:
    """Host-side schedule shared by all cores (SPMD) + per-core data."""

    def __init__(self, N, E, B, dims, src, dst, graph_ids,
                 bank=32768, st_n=768, win=256):
        self.N, self.E, self.B = N, E, B
        self.dims = dims
        self.bank, self.st_n, self.win = bank, st_n, win
        LOC = N // NCORES
        self.LOC = LOC
        self.NCH = _cdiv(LOC, P)              # node chunks per core
        self.LOCP = self.NCH * P
        self.NST = _cdiv(self.LOCP, st_n)     # supertiles per core
        self.NBANK = _cdiv(N, bank)
        self.NWIN = _cdiv(st_n, win)

        src = np.asarray(src).astype(np.int64)
        dst = np.asarray(dst).astype(np.int64)
        core = dst // LOC
        # per (core, st, bank, win): edge lists
        per_core = []
        cnt = np.zeros((NCORES, self.NST, self.NBANK, self.NWIN), np.int64)
        for c in range(NCORES):
            sel = np.nonzero(core == c)[0]
            dl = dst[sel] - c * LOC
            st = dl // st_n
            bk = src[sel] // bank
            wi = (dl % st_n) // win
            order = np.lexsort((dl, wi, bk, st))
            sel, dl, st, bk, wi = sel[order], dl[order], st[order], bk[order], wi[order]
            per_core.append((sel, dl, st, bk, wi))
            for s in range(self.NST):
                m1 = st == s
                for b in range(self.NBANK):
                    m2 = m1 & (bk == b)
                    for w in range(self.NWIN):
                        cnt[c, s, b, w] = np.count_nonzero(m2 & (wi == w))
        # common chunk counts per (st, bank, win) = max over cores, padded to 128
        self.nch_sbw = _cdiv(cnt.max(axis=0), P)          # [NST, NBANK, NWIN] chunks
        # chunk schedule (global, shared): list of (st, bank, win)
        sched = []
        for s in range(self.NST):
            for b in range(self.NBANK):
                for w in range(self.NWIN):
                    sched += [(s, b, w)] * int(self.nch_sbw[s, b, w])
        self.sched = sched
        self.TOT = len(sched)
        # per-core idx16 / dstrel arrays
        self.idx16 = np.zeros((NCORES, P, self.TOT * 8), np.int16)
        self.dstrel = np.full((NCORES, P, self.TOT), 1000.0, np.float32)
        for c in range(NCORES):
            sel, dl, st, bk, wi = per_core[c]
            gk = 0
            pos = 0  # cursor into this core's sorted edges
            for s in range(self.NST):
                for b in range(self.NBANK):
                    for w in range(self.NWIN):
                        nch = int(self.nch_sbw[s, b, w])
                        if nch == 0:
                            continue
                        ne = int(cnt[c, s, b, w])
                        e = sel[pos:pos + ne]
                        reldst = (dl[pos:pos + ne] - s * st_n - w * win)
                        pos += ne
                        nslot = nch * P
                        idx = np.zeros(nslot, np.int64)  # pad -> row 0 of bank
                        idx[:ne] = src[e] - b * bank
                        dr = np.full(nslot, 1000.0, np.float64)
                        dr[:ne] = reldst
                        # slot i -> (partition i%128, chunk i//128)
                        for k in range(nch):
                            blk = idx[k * P:(k + 1) * P]
                            self.idx16[c, :16, (gk + k) * 8:(gk + k) * 8 + 8] = \
                                blk.astype(np.int16).reshape(8, 16).T
                            self.dstrel[c, :, gk + k] = dr[k * P:(k + 1) * P].astype(np.float32)
                        gk += nch
            assert pos == len(sel)
            # replicate idx across the 8 16-partition groups
            for r in range(1, 8):
                self.idx16[c, 16 * r:16 * (r + 1), :] = self.idx16[c, :16, :]

        # per-st gather call layout: chunks for st occupy [st_c0[s], st_c0[s+1])
        self.st_c0 = np.zeros(self.NST + 1, np.int64)
        for i, (s, b, w) in enumerate(sched):
            self.st_c0[s + 1] = i + 1
        for s in range(self.NST):
            self.st_c0[s + 1] = max(self.st_c0[s + 1], self.st_c0[s])
        # per (st, bank): chunk range within schedule
        self.calls = []  # list per st of (bank, c0, nch)
        for s in range(self.NST):
            lst = []
            c0 = int(self.st_c0[s])
            for b in range(self.NBANK):
                nb = int(self.nch_sbw[s, b, :].sum())
                if nb:
                    lst.append((b, c0, nb))
                    c0 += nb
            assert c0 == int(self.st_c0[s + 1])
            self.calls.append(lst)


def _leaky(x):
    return np.maximum(x, SLOPE * x)


def build_kernel(plan):
    N, LOC, dims = plan.N, plan.LOC, plan.dims
    NCH, NST, TOT = plan.NCH, plan.NST, plan.TOT
    ST_N, WIN = plan.st_n, plan.win
    NL = len(dims) - 1
    d_last = dims[NL]
    nc = bacc.Bacc("TRN2", target_bir_lowering=False, debug=False,
                   enable_asserts=False, num_devices=NCORES)
    f32, bf16, i16 = mybir.dt.float32, mybir.dt.bfloat16, mybir.dt.int16

    h0 = nc.dram_tensor("h0", [N, _pd(dims[0])], bf16, kind="ExternalInput").ap()
    idx_in = nc.dram_tensor("idx16", [P, TOT * 8], i16, kind="ExternalInput").ap()
    dstrel_in = nc.dram_tensor("dstrel", [P, TOT], f32, kind="ExternalInput").ap()
    iota_in = nc.dram_tensor("iota", [P, WIN], bf16, kind="ExternalInput").ap()
    scol_in = nc.dram_tensor("scol", [P, NCH * 4], f32, kind="ExternalInput").ap()
    invi_in = nc.dram_tensor("invi", [1, NCH * P], bf16, kind="ExternalInput").ap()
    gw_in = nc.dram_tensor("gw", [P, NCH * 64], f32, kind="ExternalInput").ap()
    w_ins, b_ins = [], []
    for l in range(1, NL + 1):
        w_ins.append(nc.dram_tensor(f"w{l}", [dims[l - 1], dims[l]], bf16,
                                    kind="ExternalInput").ap())
        b_ins.append(nc.dram_tensor(f"b{l}", [1, dims[l]], bf16,
                                    kind="ExternalInput").ap())
    out_pool = nc.dram_tensor("pool", [64, d_last], f32, kind="ExternalOutput").ap()

    with tile.TileContext(nc) as tc:
        with tc.tile_pool(name="const", bufs=1) as cp, \
             tc.tile_pool(name="xp", bufs=2) as xp, \
             tc.tile_pool(name="ohp", bufs=8) as ohp, \
             tc.tile_pool(name="wk", bufs=3) as wk, \
             tc.tile_pool(name="aggp", bufs=2) as aggp, \
             tc.tile_pool(name="ps_agg", bufs=1, space="PSUM") as ps_agg, \
             tc.tile_pool(name="ps_out", bufs=2, space="PSUM") as ps_out, \
             tc.tile_pool(name="ps_pool", bufs=1, space="PSUM") as ps_pool, \
             tc.tile_pool(name="dram", bufs=1, space="DRAM") as dram:

            # resident constants
            idx_t = cp.tile([P, TOT * 8], i16, tag="idx", name="idx_t")
            nc.sync.dma_start(idx_t[:], idx_in[:])
            dstrel_t = cp.tile([P, TOT], f32, tag="dstrel", name="dstrel_t")
            nc.sync.dma_start(dstrel_t[:], dstrel_in[:])
            iota_t = cp.tile([P, WIN], bf16, tag="iota", name="iota_t")
            nc.sync.dma_start(iota_t[:], iota_in[:])
            scol_t = cp.tile([P, NCH * 4], f32, tag="scol", name="scol_t")
            nc.sync.dma_start(scol_t[:], scol_in[:])
            invi_t = cp.tile([1, NCH * P], bf16, tag="invi", name="invi_t")
            nc.sync.dma_start(invi_t[:], invi_in[:])
            gw_t = cp.tile([P, NCH * 64], f32, tag="gw", name="gw_t")
            nc.sync.dma_start(gw_t[:], gw_in[:])
            zz = cp.tile([1, 512], bf16, tag="zz", name="zz")
            nc.vector.memset(zz[:], 0.0)
            zzl = cp.tile([1, P], bf16, tag="zzl", name="zzl")
            nc.vector.memset(zzl[:], 0.0)
            w_hi, w_lo, b_ts = [], [], []
            for l in range(1, NL + 1):
                di, do = dims[l - 1], dims[l]
                hi = cp.tile([min(di, P), do], bf16, tag=f"wh{l}", name=f"wh{l}")
                nc.sync.dma_start(hi[:], w_ins[l - 1][0:min(di, P), :])
                w_hi.append(hi)
                if di > P:
                    lo = cp.tile([di - P, do], bf16, tag=f"wl{l}", name=f"wl{l}")
                    nc.sync.dma_start(lo[:], w_ins[l - 1][P:di, :])
                    w_lo.append(lo)
                else:
                    w_lo.append(None)
                bt = cp.tile([1, do], bf16, tag=f"b{l}", name=f"bt{l}")
                nc.sync.dma_start(bt[:], b_ins[l - 1][:])
                b_ts.append(bt)

            # AllGather buffers per layer 1..NL-1 outputs
            agi, ago = [], []
            for l in range(1, NL):
                pdo = _pd(dims[l])
                agi.append(dram.tile([LOC, pdo], bf16, tag=f"agi{l}", name=f"agi{l}"))
                ago.append(dram.tile([N, pdo], bf16, tag=f"ago{l}", name=f"ago{l}",
                                     addr_space="Shared"))

            pool_ps = ps_pool.tile([64, d_last], f32, tag="poolps", name="pool_ps")

            prev = h0
            for l in range(1, NL + 1):
                di, do = dims[l - 1], dims[l]
                pdi = _pd(di)
                dk = min(di, P)
                scol_off = (2 if l == NL else 0) * NCH
                for s in range(NST):
                    stw = min(ST_N, plan.LOCP - s * ST_N)
                    c0s = int(plan.st_c0[s])
                    nch_st = int(plan.st_c0[s + 1]) - c0s
                    if nch_st == 0:
                        continue
                    X = xp.tile([P, nch_st * pdi], bf16, tag="X", name="X")
                    X3 = X[:].rearrange("p (c d) -> p c d", d=pdi)
                    for (b, c0, nb) in plan.calls[s]:
                        b1 = min((b + 1) * plan.bank, N)
                        nc.gpsimd.dma_gather(
                            X3[:, c0 - c0s:c0 - c0s + nb, :],
                            prev[b * plan.bank:b1, :],
                            idx_t[:, c0 * 8:(c0 + nb) * 8],
                            nb * P, nb * P, pdi, elem_step=pdi,
                            single_packet=False)
                    hi_ps = ps_agg.tile([P, ST_N], f32, tag="agghi", name="hi_ps")
                    lo_ps = ps_agg.tile([P, ST_N], f32, tag="agglo", name="lo_ps") if di > P else None
                    # zero-fill PSUM via matmuls (start=True)
                    for z0 in range(0, stw, 512):
                        zn = min(512, stw - z0)
                        nc.tensor.matmul(hi_ps[:, z0:z0 + zn], lhsT=zzl[0:1, 0:P],
                                         rhs=zz[0:1, 0:zn], start=True, stop=False,
                                         skip_group_check=True)
                        if lo_ps is not None:
                            nc.tensor.matmul(lo_ps[:, z0:z0 + zn], lhsT=zzl[0:1, 0:P],
                                             rhs=zz[0:1, 0:zn], start=True, stop=False,
                                             skip_group_check=True)
                    for k in range(nch_st):
                        gk = c0s + k
                        _, _, wpos = plan.sched[gk]
                        wb = wpos * WIN
                        wn = min(WIN, stw - wb)
                        oh = ohp.tile([P, WIN], bf16, tag="oh", name="oh")
                        nc.vector.tensor_scalar(
                            out=oh[:, 0:wn], in0=iota_t[:, 0:wn],
                            scalar1=dstrel_t[:, gk:gk + 1], scalar2=None,
                            op0=mybir.AluOpType.is_equal)
                        last = k == nch_st - 1
                        nc.tensor.matmul(hi_ps[0:dk, wb:wb + wn],
                                         lhsT=X3[:, k, 0:dk], rhs=oh[:, 0:wn],
                                         start=False, stop=last,
                                         skip_group_check=True)
                        if lo_ps is not None:
                            nc.tensor.matmul(lo_ps[0:di - P, wb:wb + wn],
                                             lhsT=X3[:, k, P:di], rhs=oh[:, 0:wn],
                                             start=False, stop=last,
                                             skip_group_check=True)
                    aggh = aggp.tile([P, ST_N], bf16, tag="aggh", name="aggh")
                    nc.scalar.activation(aggh[0:dk, 0:stw], hi_ps[0:dk, 0:stw],
                                         mybir.ActivationFunctionType.Copy)
                    if lo_ps is not None:
                        aggl = aggp.tile([P, ST_N], bf16, tag="aggl", name="aggl")
                        nc.scalar.activation(aggl[0:di - P, 0:stw],
                                             lo_ps[0:di - P, 0:stw],
                                             mybir.ActivationFunctionType.Copy)
                    for nj in range(stw // P):
                        jj = (s * ST_N) // P + nj
                        nsl = slice(nj * P, (nj + 1) * P)
                        out2 = ps_out.tile([P, do], f32, tag="out2", name="out2")
                        nc.tensor.matmul(out2[:], lhsT=aggh[0:dk, nsl],
                                         rhs=w_hi[l - 1][:], start=True, stop=False,
                                         skip_group_check=True)
                        if di > P:
                            nc.tensor.matmul(out2[:], lhsT=aggl[0:di - P, nsl],
                                             rhs=w_lo[l - 1][:], start=False,
                                             stop=False, skip_group_check=True)
                        nc.tensor.matmul(out2[:], lhsT=invi_t[0:1, jj * P:(jj + 1) * P],
                                         rhs=b_ts[l - 1][:], start=False, stop=True,
                                         skip_group_check=True)
                        t = wk.tile([P, do], f32, tag="t", name="t_t")
                        nc.vector.tensor_scalar(
                            out=t[:], in0=out2[:],
                            scalar1=scol_t[:, scol_off + jj:scol_off + jj + 1],
                            scalar2=None, op0=mybir.AluOpType.mult)
                        u = wk.tile([P, do], f32, tag="u", name="u_t")
                        nc.scalar.activation(
                            u[:], out2[:], mybir.ActivationFunctionType.Copy,
                            scale=scol_t[:, scol_off + NCH + jj:scol_off + NCH + jj + 1])
                        if l < NL:
                            hp = wk.tile([P, do], bf16, tag="hp", name="hp")
                            nc.vector.tensor_tensor(out=hp[:], in0=t[:], in1=u[:],
                                                    op=mybir.AluOpType.max)
                            r0 = jj * P
                            r1 = min(r0 + P, LOC)
                            if r1 > r0:
                                nc.sync.dma_start(agi[l - 1][r0:r1, 0:do],
                                                  hp[0:r1 - r0, :])
                        else:
                            hp32 = wk.tile([P, do], f32, tag="hp32", name="hp32")
                            nc.vector.tensor_tensor(out=hp32[:], in0=t[:], in1=u[:],
                                                    op=mybir.AluOpType.max)
                            nc.tensor.matmul(pool_ps[:],
                                             lhsT=gw_t[:, jj * 64:(jj + 1) * 64],
                                             rhs=hp32[:], start=jj == 0,
                                             stop=jj == NCH - 1,
                                             skip_group_check=True)
                if l < NL:
                    nc.gpsimd.collective_compute(
                        "AllGather", mybir.AluOpType.bypass,
                        replica_groups=[list(range(NCORES))],
                        ins=[agi[l - 1][:]], outs=[ago[l - 1][:]])
                    prev = ago[l - 1]
            pool_sb = wk.tile([64, d_last], f32, tag="poolsb", name="pool_sb")
            nc.scalar.activation(pool_sb[:], pool_ps[:],
                                 mybir.ActivationFunctionType.Copy)
            nc.sync.dma_start(out_pool[:], pool_sb[:])
    nc.compile()
    return nc


def host_prep(inputs, src, dst, graph_ids, dims, N, E, B):
    """Build plan + per-core input maps."""
    src = np.asarray(src).astype(np.int64)
    dst = np.asarray(dst).astype(np.int64)
    gid = np.asarray(graph_ids).astype(np.int64)
    plan = Plan(N, E, B, dims, src, dst, gid)
    out_deg = np.maximum(np.bincount(src, minlength=N), 1).astype(np.float32)
    in_deg = np.maximum(np.bincount(dst, minlength=N), 1).astype(np.float32)
    o = out_deg ** -0.5
    i = in_deg ** -0.5
    s10 = (i * o).astype(np.float32)
    s11 = i.astype(np.float32)
    LOC, NCH = plan.LOC, plan.NCH
    scol = np.zeros((NCORES, P, NCH * 4), np.float32)
    invi = np.zeros((NCORES, 1, NCH * P), BF)
    gw = np.zeros((NCORES, P, NCH * 64), np.float32)
    for c in range(NCORES):
        loc_n = np.arange(LOC) + c * LOC
        for j in range(NCH):
            r0, r1 = j * P, min((j + 1) * P, LOC)
            npart = r1 - r0
            scol[c, 0:npart, j] = s10[loc_n[r0:r1]]
            scol[c, 0:npart, NCH + j] = SLOPE * s10[loc_n[r0:r1]]
            scol[c, 0:npart, 2 * NCH + j] = s11[loc_n[r0:r1]]
            scol[c, 0:npart, 3 * NCH + j] = SLOPE * s11[loc_n[r0:r1]]
            invi[c, 0, j * P:j * P + npart] = (1.0 / i[loc_n[r0:r1]]).astype(BF)
            g = gid[loc_n[r0:r1]]
            gw[c, np.arange(npart), j * 64 + g] = 1.0
    iota = np.tile(np.arange(plan.win, dtype=np.float32), (P, 1)).astype(BF)
    h0 = np.zeros((N, _pd(dims[0])), BF)
    h0[:, 0:dims[0]] = (np.asarray(inputs, np.float32) * o[:, None]).astype(BF)
    return plan, dict(scol=scol, invi=invi, gw=gw, iota=iota, h0=h0)


def kernel(inputs, edata, src, dst, graph_ids, conv_weights, conv_biases,
           w1, b1, w2, b2, param_mu, param_sigma):
    import jax
    N, E, B = N_FULL, E_FULL, B_FULL
    dims = DIMS_FULL
    inputs = np.asarray(inputs, np.float32)
    plan, aux = host_prep(inputs, src, dst, graph_ids, dims, N, E, B)
    nc = build_kernel(plan)
    in_maps = []
    wb = {}
    for l in range(1, len(dims)):
        wb[f"w{l}"] = np.asarray(conv_weights[l - 1], np.float32).astype(BF)
        wb[f"b{l}"] = np.asarray(conv_biases[l - 1], np.float32).reshape(1, -1).astype(BF)
    for c in range(NCORES):
        m = dict(h0=aux["h0"], iota=aux["iota"],
                 idx16=plan.idx16[c], dstrel=plan.dstrel[c],
                 scol=aux["scol"][c], invi=aux["invi"][c], gw=aux["gw"][c])
        m.update(wb)
        in_maps.append(m)
    import time
    trace = os.environ.get("GCN_TRACE") == "1"
    try:
        res = bass_utils.run_bass_kernel_spmd(nc, in_maps,
                                              core_ids=list(range(NCORES)),
                                              trace=trace)
    except ModuleNotFoundError:
        # NTFF profiling hook unavailable in this environment
        res = bass_utils.run_bass_kernel_spmd(nc, in_maps,
                                              core_ids=list(range(NCORES)))
    global last_exec_time_ns
    last_exec_time_ns = res.exec_time_ns
    if last_exec_time_ns is None:
        # time a second dispatch (NEFF already built/loaded) as an upper bound
        t0 = time.time()
        res = bass_utils.run_bass_kernel_spmd(nc, in_maps,
                                              core_ids=list(range(NCORES)))
        last_exec_time_ns = int((time.time() - t0) * 1e9)
    pool = np.zeros((64, dims[-1]), np.float64)
    for c in range(NCORES):
        pool += res.results[c]["pool"].astype(np.float64)
    gid = np.asarray(graph_ids).astype(np.int64)
    counts = np.maximum(np.bincount(gid, minlength=B), 1).astype(np.float32)
    hg = (pool.astype(np.float32)) / counts[:, None]
    hg = _leaky(hg)
    hg = hg @ np.asarray(w1, np.float32) + np.asarray(b1, np.float32)
    cpu = jax.local_devices(backend="cpu")[0]
    with jax.default_device(cpu):
        keep1 = np.asarray(jax.random.bernoulli(jax.random.key(101), 0.7, hg.shape))
        keep2 = np.asarray(jax.random.bernoulli(jax.random.key(102), 0.8,
                                                (hg.shape[0], np.asarray(w2).shape[1])))
    hg = np.where(keep1, hg / 0.7, 0.0).astype(np.float32)
    hg = _leaky(hg)
    hg = hg @ np.asarray(w2, np.float32) + np.asarray(b2, np.float32)
    hg = np.where(keep2, hg / 0.8, 0.0).astype(np.float32)
    out = 1.0 / (1.0 + np.exp(-hg))
    return out.astype(np.float32)


# revision 12
# speedup vs baseline: 74.2877x; 74.2877x over previous
"""Trainium2 Bass kernel for nn_GCN_12575664243073 (11-layer GCN + mean-pool + MLP).

Strategy (8 NeuronCores, SPMD):
  - Nodes sharded contiguously by dst across 8 cores (12500 each); small weights
    replicated; full node-feature array re-replicated each layer via AllGather.
  - Per layer: neighbor features fetched with dma_gather (int16 indices, 4
    32768-row banks), aggregated per 256-node window with one-hot matmuls on
    the PE into PSUM (exact segment-sum), then W-matmul + bias + leaky-relu
    with all per-node norms folded into matmul operands.
  - Graph mean-pool folded into layer 11 as per-chunk one-hot matmuls.
  - Tiny 64-row MLP tail (with jax dropout masks) computed on host.
"""
import os
import sys
sys.path.insert(0, "/opt/trn_rl_repo")
import numpy as np
import ml_dtypes

last_exec_time_ns = None
last_nc = None
last_in_maps = None

import concourse.bass as bass
import concourse.mybir as mybir
import concourse.bacc as bacc
import concourse.tile as tile
from concourse import bass_utils

BF = ml_dtypes.bfloat16
P = 128
NCORES = 8
SLOPE = 0.01

# problem constants (hardcoded; kernel.py must be self-contained)
N_FULL = 100000
E_FULL = 450000
B_FULL = 64
DIMS_FULL = [128, 80, 160, 112, 160, 176, 96, 144, 96, 128, 96, 160]


def _cdiv(a, b):
    return -(-a // b)


def _pd(d):
    # bf16 gather rows must be a multiple of 128 elems (256B)
    return _cdiv(d, 128) * 128


class Plan:
    """Host-side schedule shared by all cores (SPMD) + per-core data."""

    def __init__(self, N, E, B, dims, src, dst, graph_ids,
                 bank=32768, st_n=768, win=256):
        self.N, self.E, self.B = N, E, B
        self.dims = dims
        self.bank, self.st_n, self.win = bank, st_n, win
        LOC = N // NCORES
        self.LOC = LOC
        self.NCH = _cdiv(LOC, P)              # node chunks per core
        self.LOCP = self.NCH * P
        self.NST = _cdiv(self.LOCP, st_n)     # supertiles per core
        self.NBANK = _cdiv(N, bank)
        self.NWIN = _cdiv(st_n, win)

        src = np.asarray(src).astype(np.int64)
        dst = np.asarray(dst).astype(np.int64)
        core = dst // LOC
        # per (core, st, bank, win): edge lists
        per_core = []
        cnt = np.zeros((NCORES, self.NST, self.NBANK, self.NWIN), np.int64)
        for c in range(NCORES):
            sel = np.nonzero(core == c)[0]
            dl = dst[sel] - c * LOC
            st = dl // st_n
            bk = src[sel] // bank
            wi = (dl % st_n) // win
            order = np.lexsort((dl, wi, bk, st))
            sel, dl, st, bk, wi = sel[order], dl[order], st[order], bk[order], wi[order]
            per_core.append((sel, dl, st, bk, wi))
            for s in range(self.NST):
                m1 = st == s
                for b in range(self.NBANK):
                    m2 = m1 & (bk == b)
                    for w in range(self.NWIN):
                        cnt[c, s, b, w] = np.count_nonzero(m2 & (wi == w))
        # common chunk counts per (st, bank, win) = max over cores, padded to 128
        self.nch_sbw = _cdiv(cnt.max(axis=0), P)          # [NST, NBANK, NWIN] chunks
        # chunk schedule (global, shared): list of (st, bank, win)
        sched = []
        for s in range(self.NST):
            for b in range(self.NBANK):
                for w in range(self.NWIN):
                    sched += [(s, b, w)] * int(self.nch_sbw[s, b, w])
        self.sched = sched
        self.TOT = len(sched)
        # per-core idx16 / dstrel arrays
        self.idx16 = np.zeros((NCORES, P, self.TOT * 8), np.int16)
        self.dstrel = np.full((NCORES, P, self.TOT), 1000.0, np.float32)
        for c in range(NCORES):
            sel, dl, st, bk, wi = per_core[c]
            gk = 0
            pos = 0  # cursor into this core's sorted edges
            for s in range(self.NST):
                for b in range(self.NBANK):
                    for w in range(self.NWIN):
                        nch = int(self.nch_sbw[s, b, w])
                        if nch == 0:
                            continue
                        ne = int(cnt[c, s, b, w])
                        e = sel[pos:pos + ne]
                        reldst = (dl[pos:pos + ne] - s * st_n - w * win)
                        pos += ne
                        nslot = nch * P
                        idx = np.zeros(nslot, np.int64)  # pad -> row 0 of bank
                        idx[:ne] = src[e] - b * bank
                        dr = np.full(nslot, 1000.0, np.float64)
                        dr[:ne] = reldst
                        # slot i -> (partition i%128, chunk i//128)
                        for k in range(nch):
                            blk = idx[k * P:(k + 1) * P]
                            self.idx16[c, :16, (gk + k) * 8:(gk + k) * 8 + 8] = \
                                blk.astype(np.int16).reshape(8, 16).T
                            self.dstrel[c, :, gk + k] = dr[k * P:(k + 1) * P].astype(np.float32)
                        gk += nch
            assert pos == len(sel)
            # replicate idx across the 8 16-partition groups
            for r in range(1, 8):
                self.idx16[c, 16 * r:16 * (r + 1), :] = self.idx16[c, :16, :]

        # per-st gather call layout: chunks for st occupy [st_c0[s], st_c0[s+1])
        self.st_c0 = np.zeros(self.NST + 1, np.int64)
        for i, (s, b, w) in enumerate(sched):
            self.st_c0[s + 1] = i + 1
        for s in range(self.NST):
            self.st_c0[s + 1] = max(self.st_c0[s + 1], self.st_c0[s])
        # per (st, bank): chunk range within schedule
        self.calls = []  # list per st of (bank, c0, nch)
        for s in range(self.NST):
            lst = []
            c0 = int(self.st_c0[s])
            for b in range(self.NBANK):
                nb = int(self.nch_sbw[s, b, :].sum())
                if nb:
                    lst.append((b, c0, nb))
                    c0 += nb
            assert c0 == int(self.st_c0[s + 1])
            self.calls.append(lst)


def _leaky(x):
    return np.maximum(x, SLOPE * x)


def build_kernel(plan):
    N, LOC, dims = plan.N, plan.LOC, plan.dims
    NCH, NST, TOT = plan.NCH, plan.NST, plan.TOT
    ST_N, WIN = plan.st_n, plan.win
    NL = len(dims) - 1
    d_last = dims[NL]
    nc = bacc.Bacc("TRN2", target_bir_lowering=False, debug=False,
                   enable_asserts=False, num_devices=NCORES)
    f32, bf16, i16 = mybir.dt.float32, mybir.dt.bfloat16, mybir.dt.int16

    h0 = nc.dram_tensor("h0", [N, _pd(dims[0])], bf16, kind="ExternalInput").ap()
    idx_in = nc.dram_tensor("idx16", [P, TOT * 8], i16, kind="ExternalInput").ap()
    dstrel_in = nc.dram_tensor("dstrel", [P, TOT], f32, kind="ExternalInput").ap()
    iota_in = nc.dram_tensor("iota", [P, WIN], bf16, kind="ExternalInput").ap()
    scol_in = nc.dram_tensor("scol", [P, NCH * 4], f32, kind="ExternalInput").ap()
    invi_in = nc.dram_tensor("invi", [1, NCH * P], bf16, kind="ExternalInput").ap()
    gw_in = nc.dram_tensor("gw", [P, NCH * 64], f32, kind="ExternalInput").ap()
    w_ins, b_ins = [], []
    for l in range(1, NL + 1):
        w_ins.append(nc.dram_tensor(f"w{l}", [dims[l - 1], dims[l]], bf16,
                                    kind="ExternalInput").ap())
        b_ins.append(nc.dram_tensor(f"b{l}", [1, dims[l]], bf16,
                                    kind="ExternalInput").ap())
    out_pool = nc.dram_tensor("pool", [64, d_last], f32, kind="ExternalOutput").ap()

    with tile.TileContext(nc) as tc:
        with tc.tile_pool(name="const", bufs=1) as cp, \
             tc.tile_pool(name="xp", bufs=2) as xp, \
             tc.tile_pool(name="ohp", bufs=8) as ohp, \
             tc.tile_pool(name="wk", bufs=3) as wk, \
             tc.tile_pool(name="aggp", bufs=2) as aggp, \
             tc.tile_pool(name="ps_agg", bufs=1, space="PSUM") as ps_agg, \
             tc.tile_pool(name="ps_out", bufs=2, space="PSUM") as ps_out, \
             tc.tile_pool(name="ps_pool", bufs=1, space="PSUM") as ps_pool, \
             tc.tile_pool(name="dram", bufs=1, space="DRAM") as dram:

            # resident constants
            idx_t = cp.tile([P, TOT * 8], i16, tag="idx", name="idx_t")
            nc.sync.dma_start(idx_t[:], idx_in[:])
            dstrel_t = cp.tile([P, TOT], f32, tag="dstrel", name="dstrel_t")
            nc.sync.dma_start(dstrel_t[:], dstrel_in[:])
            iota_t = cp.tile([P, WIN], bf16, tag="iota", name="iota_t")
            nc.sync.dma_start(iota_t[:], iota_in[:])
            scol_t = cp.tile([P, NCH * 4], f32, tag="scol", name="scol_t")
            nc.sync.dma_start(scol_t[:], scol_in[:])
            invi_t = cp.tile([1, NCH * P], bf16, tag="invi", name="invi_t")
            nc.sync.dma_start(invi_t[:], invi_in[:])
            gw_t = cp.tile([P, NCH * 64], f32, tag="gw", name="gw_t")
            nc.sync.dma_start(gw_t[:], gw_in[:])
            zz = cp.tile([1, 512], bf16, tag="zz", name="zz")
            nc.vector.memset(zz[:], 0.0)
            zzl = cp.tile([1, P], bf16, tag="zzl", name="zzl")
            nc.vector.memset(zzl[:], 0.0)
            w_hi, w_lo, b_ts = [], [], []
            for l in range(1, NL + 1):
                di, do = dims[l - 1], dims[l]
                hi = cp.tile([min(di, P), do], bf16, tag=f"wh{l}", name=f"wh{l}")
                nc.sync.dma_start(hi[:], w_ins[l - 1][0:min(di, P), :])
                w_hi.append(hi)
                if di > P:
                    lo = cp.tile([di - P, do], bf16, tag=f"wl{l}", name=f"wl{l}")
                    nc.sync.dma_start(lo[:], w_ins[l - 1][P:di, :])
                    w_lo.append(lo)
                else:
                    w_lo.append(None)
                bt = cp.tile([1, do], bf16, tag=f"b{l}", name=f"bt{l}")
                nc.sync.dma_start(bt[:], b_ins[l - 1][:])
                b_ts.append(bt)

            # AllGather buffers per layer 1..NL-1 outputs
            agi, ago = [], []
            for l in range(1, NL):
                pdo = _pd(dims[l])
                agi.append(dram.tile([LOC, pdo], bf16, tag=f"agi{l}", name=f"agi{l}"))
                ago.append(dram.tile([N, pdo], bf16, tag=f"ago{l}", name=f"ago{l}",
                                     addr_space="Shared"))

            pool_ps = ps_pool.tile([64, d_last], f32, tag="poolps", name="pool_ps")

            prev = h0
            for l in range(1, NL + 1):
                di, do = dims[l - 1], dims[l]
                pdi = _pd(di)
                dk = min(di, P)
                scol_off = (2 if l == NL else 0) * NCH
                for s in range(NST):
                    stw = min(ST_N, plan.LOCP - s * ST_N)
                    c0s = int(plan.st_c0[s])
                    nch_st = int(plan.st_c0[s + 1]) - c0s
                    if nch_st == 0:
                        continue
                    X = xp.tile([P, nch_st * pdi], bf16, tag="X", name="X")
                    X3 = X[:].rearrange("p (c d) -> p c d", d=pdi)
                    for (b, c0, nb) in plan.calls[s]:
                        b1 = min((b + 1) * plan.bank, N)
                        nc.gpsimd.dma_gather(
                            X3[:, c0 - c0s:c0 - c0s + nb, :],
                            prev[b * plan.bank:b1, :],
                            idx_t[:, c0 * 8:(c0 + nb) * 8],
                            nb * P, nb * P, pdi, elem_step=pdi,
                            single_packet=False)
                    hi_ps = ps_agg.tile([P, ST_N], f32, tag="agghi", name="hi_ps")
                    lo_ps = ps_agg.tile([P, ST_N], f32, tag="agglo", name="lo_ps") if di > P else None
                    # zero-fill PSUM via matmuls (start=True)
                    for z0 in range(0, stw, 512):
                        zn = min(512, stw - z0)
                        nc.tensor.matmul(hi_ps[:, z0:z0 + zn], lhsT=zzl[0:1, 0:P],
                                         rhs=zz[0:1, 0:zn], start=True, stop=False,
                                         skip_group_check=True)
                        if lo_ps is not None:
                            nc.tensor.matmul(lo_ps[:, z0:z0 + zn], lhsT=zzl[0:1, 0:P],
                                             rhs=zz[0:1, 0:zn], start=True, stop=False,
                                             skip_group_check=True)
                    for k in range(nch_st):
                        gk = c0s + k
                        _, _, wpos = plan.sched[gk]
                        wb = wpos * WIN
                        wn = min(WIN, stw - wb)
                        oh = ohp.tile([P, WIN], bf16, tag="oh", name="oh")
                        nc.vector.tensor_scalar(
                            out=oh[:, 0:wn], in0=iota_t[:, 0:wn],
                            scalar1=dstrel_t[:, gk:gk + 1], scalar2=None,
                            op0=mybir.AluOpType.is_equal)
                        last = k == nch_st - 1
                        nc.tensor.matmul(hi_ps[0:dk, wb:wb + wn],
                                         lhsT=X3[:, k, 0:dk], rhs=oh[:, 0:wn],
                                         start=False, stop=last,
                                         skip_group_check=True)
                        if lo_ps is not None:
                            nc.tensor.matmul(lo_ps[0:di - P, wb:wb + wn],
                                             lhsT=X3[:, k, P:di], rhs=oh[:, 0:wn],
                                             start=False, stop=last,
                                             skip_group_check=True)
                    aggh = aggp.tile([P, ST_N], bf16, tag="aggh", name="aggh")
                    nc.scalar.activation(aggh[0:dk, 0:stw], hi_ps[0:dk, 0:stw],
                                         mybir.ActivationFunctionType.Copy)
                    if lo_ps is not None:
                        aggl = aggp.tile([P, ST_N], bf16, tag="aggl", name="aggl")
                        nc.scalar.activation(aggl[0:di - P, 0:stw],
                                             lo_ps[0:di - P, 0:stw],
                                             mybir.ActivationFunctionType.Copy)
                    for nj in range(stw // P):
                        jj = (s * ST_N) // P + nj
                        nsl = slice(nj * P, (nj + 1) * P)
                        out2 = ps_out.tile([P, do], f32, tag="out2", name="out2")
                        nc.tensor.matmul(out2[:], lhsT=aggh[0:dk, nsl],
                                         rhs=w_hi[l - 1][:], start=True, stop=False,
                                         skip_group_check=True)
                        if di > P:
                            nc.tensor.matmul(out2[:], lhsT=aggl[0:di - P, nsl],
                                             rhs=w_lo[l - 1][:], start=False,
                                             stop=False, skip_group_check=True)
                        nc.tensor.matmul(out2[:], lhsT=invi_t[0:1, jj * P:(jj + 1) * P],
                                         rhs=b_ts[l - 1][:], start=False, stop=True,
                                         skip_group_check=True)
                        t = wk.tile([P, do], f32, tag="t", name="t_t")
                        nc.vector.tensor_scalar(
                            out=t[:], in0=out2[:],
                            scalar1=scol_t[:, scol_off + jj:scol_off + jj + 1],
                            scalar2=None, op0=mybir.AluOpType.mult)
                        u = wk.tile([P, do], f32, tag="u", name="u_t")
                        nc.scalar.activation(
                            u[:], out2[:], mybir.ActivationFunctionType.Copy,
                            scale=scol_t[:, scol_off + NCH + jj:scol_off + NCH + jj + 1])
                        if l < NL:
                            hp = wk.tile([P, do], bf16, tag="hp", name="hp")
                            nc.vector.tensor_tensor(out=hp[:], in0=t[:], in1=u[:],
                                                    op=mybir.AluOpType.max)
                            r0 = jj * P
                            r1 = min(r0 + P, LOC)
                            if r1 > r0:
                                nc.sync.dma_start(agi[l - 1][r0:r1, 0:do],
                                                  hp[0:r1 - r0, :])
                        else:
                            hp32 = wk.tile([P, do], f32, tag="hp32", name="hp32")
                            nc.vector.tensor_tensor(out=hp32[:], in0=t[:], in1=u[:],
                                                    op=mybir.AluOpType.max)
                            nc.tensor.matmul(pool_ps[:],
                                             lhsT=gw_t[:, jj * 64:(jj + 1) * 64],
                                             rhs=hp32[:], start=jj == 0,
                                             stop=jj == NCH - 1,
                                             skip_group_check=True)
                if l < NL:
                    nc.gpsimd.collective_compute(
                        "AllGather", mybir.AluOpType.bypass,
                        replica_groups=[list(range(NCORES))],
                        ins=[agi[l - 1][:]], outs=[ago[l - 1][:]])
                    prev = ago[l - 1]
            pool_sb = wk.tile([64, d_last], f32, tag="poolsb", name="pool_sb")
            nc.scalar.activation(pool_sb[:], pool_ps[:],
                                 mybir.ActivationFunctionType.Copy)
            nc.sync.dma_start(out_pool[:], pool_sb[:])
    nc.compile()
    return nc


def host_prep(inputs, src, dst, graph_ids, dims, N, E, B):
    """Build plan + per-core input maps."""
    src = np.asarray(src).astype(np.int64)
    dst = np.asarray(dst).astype(np.int64)
    gid = np.asarray(graph_ids).astype(np.int64)
    plan = Plan(N, E, B, dims, src, dst, gid)
    out_deg = np.maximum(np.bincount(src, minlength=N), 1).astype(np.float32)
    in_deg = np.maximum(np.bincount(dst, minlength=N), 1).astype(np.float32)
    o = out_deg ** -0.5
    i = in_deg ** -0.5
    s10 = (i * o).astype(np.float32)
    s11 = i.astype(np.float32)
    LOC, NCH = plan.LOC, plan.NCH
    scol = np.zeros((NCORES, P, NCH * 4), np.float32)
    invi = np.zeros((NCORES, 1, NCH * P), BF)
    gw = np.zeros((NCORES, P, NCH * 64), np.float32)
    for c in range(NCORES):
        loc_n = np.arange(LOC) + c * LOC
        for j in range(NCH):
            r0, r1 = j * P, min((j + 1) * P, LOC)
            npart = r1 - r0
            scol[c, 0:npart, j] = s10[loc_n[r0:r1]]
            scol[c, 0:npart, NCH + j] = SLOPE * s10[loc_n[r0:r1]]
            scol[c, 0:npart, 2 * NCH + j] = s11[loc_n[r0:r1]]
            scol[c, 0:npart, 3 * NCH + j] = SLOPE * s11[loc_n[r0:r1]]
            invi[c, 0, j * P:j * P + npart] = (1.0 / i[loc_n[r0:r1]]).astype(BF)
            g = gid[loc_n[r0:r1]]
            gw[c, np.arange(npart), j * 64 + g] = 1.0
    iota = np.tile(np.arange(plan.win, dtype=np.float32), (P, 1)).astype(BF)
    h0 = np.zeros((N, _pd(dims[0])), BF)
    h0[:, 0:dims[0]] = (np.asarray(inputs, np.float32) * o[:, None]).astype(BF)
    return plan, dict(scol=scol, invi=invi, gw=gw, iota=iota, h0=h0)


def kernel(inputs, edata, src, dst, graph_ids, conv_weights, conv_biases,
           w1, b1, w2, b2, param_mu, param_sigma):
    import jax
    N, E, B = N_FULL, E_FULL, B_FULL
    dims = DIMS_FULL
    inputs = np.asarray(inputs, np.float32)
    plan, aux = host_prep(inputs, src, dst, graph_ids, dims, N, E, B)
    nc = build_kernel(plan)
    in_maps = []
    wb = {}
    for l in range(1, len(dims)):
        wb[f"w{l}"] = np.asarray(conv_weights[l - 1], np.float32).astype(BF)
        wb[f"b{l}"] = np.asarray(conv_biases[l - 1], np.float32).reshape(1, -1).astype(BF)
    for c in range(NCORES):
        m = dict(h0=aux["h0"], iota=aux["iota"],
                 idx16=plan.idx16[c], dstrel=plan.dstrel[c],
                 scol=aux["scol"][c], invi=aux["invi"][c], gw=aux["gw"][c])
        m.update(wb)
        in_maps.append(m)
    trace = os.environ.get("GCN_TRACE") == "1"
    try:
        res = bass_utils.run_bass_kernel_spmd(nc, in_maps,
                                              core_ids=list(range(NCORES)),
                                              trace=trace)
    except ModuleNotFoundError:
        # NTFF profiling hook unavailable in this environment
        res = bass_utils.run_bass_kernel_spmd(nc, in_maps,
                                              core_ids=list(range(NCORES)))
    global last_exec_time_ns, last_nc, last_in_maps
    last_exec_time_ns = res.exec_time_ns
    last_nc, last_in_maps = nc, in_maps
    pool = np.zeros((64, dims[-1]), np.float64)
    for c in range(NCORES):
        pool += res.results[c]["pool"].astype(np.float64)
    gid = np.asarray(graph_ids).astype(np.int64)
    counts = np.maximum(np.bincount(gid, minlength=B), 1).astype(np.float32)
    hg = (pool.astype(np.float32)) / counts[:, None]
    hg = _leaky(hg)
    hg = hg @ np.asarray(w1, np.float32) + np.asarray(b1, np.float32)
    cpu = jax.local_devices(backend="cpu")[0]
    with jax.default_device(cpu):
        keep1 = np.asarray(jax.random.bernoulli(jax.random.key(101), 0.7, hg.shape))
        keep2 = np.asarray(jax.random.bernoulli(jax.random.key(102), 0.8,
                                                (hg.shape[0], np.asarray(w2).shape[1])))
    hg = np.where(keep1, hg / 0.7, 0.0).astype(np.float32)
    hg = _leaky(hg)
    hg = hg @ np.asarray(w2, np.float32) + np.asarray(b2, np.float32)
    hg = np.where(keep2, hg / 0.8, 0.0).astype(np.float32)
    out = 1.0 / (1.0 + np.exp(-hg))
    return out.astype(np.float32)


# revision 16
# speedup vs baseline: 94.2588x; 1.2688x over previous
"""Trainium2 Bass kernel for nn_GCN_12575664243073 (11-layer GCN + mean-pool + MLP).

Strategy (8 NeuronCores, SPMD):
  - Nodes sharded contiguously by dst across 8 cores (12500 each); small weights
    replicated; full node-feature array re-replicated each layer via AllGather.
  - Per layer: neighbor features fetched with dma_gather (int16 indices, 4
    32768-row banks), aggregated per 256-node window with one-hot matmuls on
    the PE into PSUM (exact segment-sum), then W-matmul + bias + leaky-relu
    with all per-node norms folded into matmul operands.
  - Graph mean-pool folded into layer 11 as per-chunk one-hot matmuls.
  - Tiny 64-row MLP tail (with jax dropout masks) computed on host.
"""
import os
import sys
sys.path.insert(0, "/opt/trn_rl_repo")
import numpy as np
import ml_dtypes

last_exec_time_ns = None
last_nc = None
last_in_maps = None

import concourse.bass as bass
import concourse.mybir as mybir
import concourse.bacc as bacc
import concourse.tile as tile
from concourse import bass_utils

BF = ml_dtypes.bfloat16
P = 128
NCORES = 8
SLOPE = 0.01

# problem constants (hardcoded; kernel.py must be self-contained)
N_FULL = 100000
E_FULL = 450000
B_FULL = 64
DIMS_FULL = [128, 80, 160, 112, 160, 176, 96, 144, 96, 128, 96, 160]


def _cdiv(a, b):
    return -(-a // b)


def _pd(d):
    # bf16 gather rows must be a multiple of 128 elems (256B)
    return _cdiv(d, 128) * 128


class Plan:
    """Host-side schedule shared by all cores (SPMD) + per-core data."""

    def __init__(self, N, E, B, dims, src, dst, graph_ids,
                 bank=32768, st_n=768, win=256):
        self.N, self.E, self.B = N, E, B
        self.dims = dims
        self.bank, self.st_n, self.win = bank, st_n, win
        LOC = N // NCORES
        self.LOC = LOC
        self.NCH = _cdiv(LOC, P)              # node chunks per core
        self.LOCP = self.NCH * P
        self.NST = _cdiv(self.LOCP, st_n)     # supertiles per core
        self.NBANK = _cdiv(N, bank)
        self.NWIN = _cdiv(st_n, win)

        src = np.asarray(src).astype(np.int64)
        dst = np.asarray(dst).astype(np.int64)
        core = dst // LOC
        # per (core, st, bank, win): edge lists
        per_core = []
        cnt = np.zeros((NCORES, self.NST, self.NBANK, self.NWIN), np.int64)
        for c in range(NCORES):
            sel = np.nonzero(core == c)[0]
            dl = dst[sel] - c * LOC
            st = dl // st_n
            bk = src[sel] // bank
            wi = (dl % st_n) // win
            order = np.lexsort((dl, wi, bk, st))
            sel, dl, st, bk, wi = sel[order], dl[order], st[order], bk[order], wi[order]
            per_core.append((sel, dl, st, bk, wi))
            for s in range(self.NST):
                m1 = st == s
                for b in range(self.NBANK):
                    m2 = m1 & (bk == b)
                    for w in range(self.NWIN):
                        cnt[c, s, b, w] = np.count_nonzero(m2 & (wi == w))
        # common chunk counts per (st, bank, win) = max over cores, padded to 128
        self.nch_sbw = _cdiv(cnt.max(axis=0), P)          # [NST, NBANK, NWIN] chunks
        # chunk schedule (global, shared): list of (st, bank, win)
        sched = []
        for s in range(self.NST):
            for b in range(self.NBANK):
                for w in range(self.NWIN):
                    sched += [(s, b, w)] * int(self.nch_sbw[s, b, w])
        self.sched = sched
        self.TOT = len(sched)
        # per-core idx16 / dstrel arrays
        self.idx16 = np.zeros((NCORES, P, self.TOT * 8), np.int16)
        self.dstrel = np.full((NCORES, P, self.TOT), 1000.0, np.float32)
        for c in range(NCORES):
            sel, dl, st, bk, wi = per_core[c]
            gk = 0
            pos = 0  # cursor into this core's sorted edges
            for s in range(self.NST):
                for b in range(self.NBANK):
                    for w in range(self.NWIN):
                        nch = int(self.nch_sbw[s, b, w])
                        if nch == 0:
                            continue
                        ne = int(cnt[c, s, b, w])
                        e = sel[pos:pos + ne]
                        reldst = (dl[pos:pos + ne] - s * st_n - w * win)
                        pos += ne
                        nslot = nch * P
                        idx = np.zeros(nslot, np.int64)  # pad -> row 0 of bank
                        idx[:ne] = src[e] - b * bank
                        dr = np.full(nslot, 1000.0, np.float64)
                        dr[:ne] = reldst
                        # slot i -> (partition i%128, chunk i//128)
                        for k in range(nch):
                            blk = idx[k * P:(k + 1) * P]
                            self.idx16[c, :16, (gk + k) * 8:(gk + k) * 8 + 8] = \
                                blk.astype(np.int16).reshape(8, 16).T
                            self.dstrel[c, :, gk + k] = dr[k * P:(k + 1) * P].astype(np.float32)
                        gk += nch
            assert pos == len(sel)
            # replicate idx across the 8 16-partition groups
            for r in range(1, 8):
                self.idx16[c, 16 * r:16 * (r + 1), :] = self.idx16[c, :16, :]

        # per-st gather call layout: chunks for st occupy [st_c0[s], st_c0[s+1])
        self.st_c0 = np.zeros(self.NST + 1, np.int64)
        for i, (s, b, w) in enumerate(sched):
            self.st_c0[s + 1] = i + 1
        for s in range(self.NST):
            self.st_c0[s + 1] = max(self.st_c0[s + 1], self.st_c0[s])
        # per (st, bank): chunk range within schedule
        self.calls = []  # list per st of (bank, c0, nch)
        for s in range(self.NST):
            lst = []
            c0 = int(self.st_c0[s])
            for b in range(self.NBANK):
                nb = int(self.nch_sbw[s, b, :].sum())
                if nb:
                    lst.append((b, c0, nb))
                    c0 += nb
            assert c0 == int(self.st_c0[s + 1])
            self.calls.append(lst)


def _leaky(x):
    return np.maximum(x, SLOPE * x)


def build_kernel(plan):
    N, LOC, dims = plan.N, plan.LOC, plan.dims
    NCH, NST, TOT = plan.NCH, plan.NST, plan.TOT
    ST_N, WIN = plan.st_n, plan.win
    NL = len(dims) - 1
    d_last = dims[NL]
    nc = bacc.Bacc("TRN2", target_bir_lowering=False, debug=False,
                   enable_asserts=False, num_devices=NCORES)
    f32, bf16, i16 = mybir.dt.float32, mybir.dt.bfloat16, mybir.dt.int16

    h0 = nc.dram_tensor("h0", [N, _pd(dims[0])], bf16, kind="ExternalInput").ap()
    idx_in = nc.dram_tensor("idx16", [P, TOT * 8], i16, kind="ExternalInput").ap()
    dstrel_in = nc.dram_tensor("dstrel", [P, TOT], f32, kind="ExternalInput").ap()
    iota_in = nc.dram_tensor("iota", [P, WIN], bf16, kind="ExternalInput").ap()
    scol_in = nc.dram_tensor("scol", [P, NCH * 4], f32, kind="ExternalInput").ap()
    invi_in = nc.dram_tensor("invi", [1, NCH * P], bf16, kind="ExternalInput").ap()
    gw_in = nc.dram_tensor("gw", [P, NCH * 64], f32, kind="ExternalInput").ap()
    w_ins, b_ins = [], []
    for l in range(1, NL + 1):
        w_ins.append(nc.dram_tensor(f"w{l}", [dims[l - 1], dims[l]], bf16,
                                    kind="ExternalInput").ap())
        b_ins.append(nc.dram_tensor(f"b{l}", [1, dims[l]], bf16,
                                    kind="ExternalInput").ap())
    out_pool = nc.dram_tensor("pool", [64, d_last], f32, kind="ExternalOutput").ap()

    def _bufs(k, dflt):
        for kv in os.environ.get("GCN_BUFS", "").split(","):
            if kv.startswith(k + "="):
                return int(kv.split("=")[1])
        return dflt

    with tile.TileContext(nc) as tc:
        with tc.tile_pool(name="const", bufs=1) as cp, \
             tc.tile_pool(name="xp", bufs=_bufs("xp", 4)) as xp, \
             tc.tile_pool(name="ohp", bufs=_bufs("ohp", 24)) as ohp, \
             tc.tile_pool(name="wk", bufs=_bufs("wk", 12)) as wk, \
             tc.tile_pool(name="aggp", bufs=_bufs("aggp", 3)) as aggp, \
             tc.tile_pool(name="ps_agg", bufs=1, space="PSUM") as ps_agg, \
             tc.tile_pool(name="ps_out", bufs=_bufs("ps_out", 3), space="PSUM") as ps_out, \
             tc.tile_pool(name="ps_pool", bufs=1, space="PSUM") as ps_pool, \
             tc.tile_pool(name="dram", bufs=1, space="DRAM") as dram:

            # resident constants
            idx_t = cp.tile([P, TOT * 8], i16, tag="idx", name="idx_t")
            nc.sync.dma_start(idx_t[:], idx_in[:])
            dstrel_t = cp.tile([P, TOT], f32, tag="dstrel", name="dstrel_t")
            nc.sync.dma_start(dstrel_t[:], dstrel_in[:])
            iota_t = cp.tile([P, WIN], bf16, tag="iota", name="iota_t")
            nc.sync.dma_start(iota_t[:], iota_in[:])
            scol_t = cp.tile([P, NCH * 4], f32, tag="scol", name="scol_t")
            nc.sync.dma_start(scol_t[:], scol_in[:])
            invi_t = cp.tile([1, NCH * P], bf16, tag="invi", name="invi_t")
            nc.sync.dma_start(invi_t[:], invi_in[:])
            gw_t = cp.tile([P, NCH * 64], f32, tag="gw", name="gw_t")
            nc.sync.dma_start(gw_t[:], gw_in[:])
            zz = cp.tile([1, 512], bf16, tag="zz", name="zz")
            nc.vector.memset(zz[:], 0.0)
            zzl = cp.tile([1, P], bf16, tag="zzl", name="zzl")
            nc.vector.memset(zzl[:], 0.0)
            w_hi, w_lo, b_ts = [], [], []
            for l in range(1, NL + 1):
                di, do = dims[l - 1], dims[l]
                hi = cp.tile([min(di, P), do], bf16, tag=f"wh{l}", name=f"wh{l}")
                nc.sync.dma_start(hi[:], w_ins[l - 1][0:min(di, P), :])
                w_hi.append(hi)
                if di > P:
                    lo = cp.tile([di - P, do], bf16, tag=f"wl{l}", name=f"wl{l}")
                    nc.sync.dma_start(lo[:], w_ins[l - 1][P:di, :])
                    w_lo.append(lo)
                else:
                    w_lo.append(None)
                bt = cp.tile([1, do], bf16, tag=f"b{l}", name=f"bt{l}")
                nc.sync.dma_start(bt[:], b_ins[l - 1][:])
                b_ts.append(bt)

            # AllGather buffers per layer 1..NL-1 outputs
            agi, ago = [], []
            for l in range(1, NL):
                pdo = _pd(dims[l])
                agi.append(dram.tile([LOC, pdo], bf16, tag=f"agi{l}", name=f"agi{l}"))
                ago.append(dram.tile([N, pdo], bf16, tag=f"ago{l}", name=f"ago{l}",
                                     addr_space="Shared"))

            pool_ps = ps_pool.tile([64, d_last], f32, tag="poolps", name="pool_ps")

            prev = h0
            for l in range(1, NL + 1):
                di, do = dims[l - 1], dims[l]
                pdi = _pd(di)
                dk = min(di, P)
                scol_off = (2 if l == NL else 0) * NCH
                for s in range(NST):
                    stw = min(ST_N, plan.LOCP - s * ST_N)
                    c0s = int(plan.st_c0[s])
                    nch_st = int(plan.st_c0[s + 1]) - c0s
                    if nch_st == 0:
                        continue
                    X = xp.tile([P, nch_st * pdi], bf16, tag="X", name="X")
                    X3 = X[:].rearrange("p (c d) -> p c d", d=pdi)
                    skips = set(os.environ.get("GCN_SKIP", "").split(","))
                    for (b, c0, nb) in plan.calls[s]:
                        if "gather" in skips:
                            break
                        b1 = min((b + 1) * plan.bank, N)
                        nc.gpsimd.dma_gather(
                            X3[:, c0 - c0s:c0 - c0s + nb, :],
                            prev[b * plan.bank:b1, :],
                            idx_t[:, c0 * 8:(c0 + nb) * 8],
                            nb * P, nb * P, pdi, elem_step=pdi,
                            single_packet=False)
                    hi_ps = ps_agg.tile([P, ST_N], f32, tag="agghi", name="hi_ps")
                    lo_ps = ps_agg.tile([P, ST_N], f32, tag="agglo", name="lo_ps") if di > P else None
                    # zero-fill PSUM via matmuls (start=True)
                    for z0 in range(0, stw, 512):
                        zn = min(512, stw - z0)
                        nc.tensor.matmul(hi_ps[:, z0:z0 + zn], lhsT=zzl[0:1, 0:P],
                                         rhs=zz[0:1, 0:zn], start=True, stop=False,
                                         skip_group_check=True)
                        if lo_ps is not None:
                            nc.tensor.matmul(lo_ps[:, z0:z0 + zn], lhsT=zzl[0:1, 0:P],
                                             rhs=zz[0:1, 0:zn], start=True, stop=False,
                                             skip_group_check=True)
                    for k in range(nch_st):
                        gk = c0s + k
                        _, _, wpos = plan.sched[gk]
                        wb = wpos * WIN
                        wn = min(WIN, stw - wb)
                        oh = ohp.tile([P, WIN], bf16, tag="oh", name="oh")
                        if "oh" not in skips:
                            nc.vector.tensor_scalar(
                            out=oh[:, 0:wn], in0=iota_t[:, 0:wn],
                            scalar1=dstrel_t[:, gk:gk + 1], scalar2=None,
                            op0=mybir.AluOpType.is_equal)
                        last = k == nch_st - 1
                        if "aggmm" in skips:
                            continue
                        nc.tensor.matmul(hi_ps[0:dk, wb:wb + wn],
                                         lhsT=X3[:, k, 0:dk], rhs=oh[:, 0:wn],
                                         start=False, stop=last,
                                         skip_group_check=True)
                        if lo_ps is not None:
                            nc.tensor.matmul(lo_ps[0:di - P, wb:wb + wn],
                                             lhsT=X3[:, k, P:di], rhs=oh[:, 0:wn],
                                             start=False, stop=last,
                                             skip_group_check=True)
                    aggh = aggp.tile([P, ST_N], bf16, tag="aggh", name="aggh")
                    nc.scalar.activation(aggh[0:dk, 0:stw], hi_ps[0:dk, 0:stw],
                                         mybir.ActivationFunctionType.Copy)
                    if lo_ps is not None:
                        aggl = aggp.tile([P, ST_N], bf16, tag="aggl", name="aggl")
                        nc.scalar.activation(aggl[0:di - P, 0:stw],
                                             lo_ps[0:di - P, 0:stw],
                                             mybir.ActivationFunctionType.Copy)
                    for nj in range(stw // P):
                        jj = (s * ST_N) // P + nj
                        nsl = slice(nj * P, (nj + 1) * P)
                        out2 = ps_out.tile([P, do], f32, tag="out2", name="out2")
                        nc.tensor.matmul(out2[:], lhsT=aggh[0:dk, nsl],
                                         rhs=w_hi[l - 1][:], start=True, stop=False,
                                         skip_group_check=True)
                        if di > P:
                            nc.tensor.matmul(out2[:], lhsT=aggl[0:di - P, nsl],
                                             rhs=w_lo[l - 1][:], start=False,
                                             stop=False, skip_group_check=True)
                        nc.tensor.matmul(out2[:], lhsT=invi_t[0:1, jj * P:(jj + 1) * P],
                                         rhs=b_ts[l - 1][:], start=False, stop=True,
                                         skip_group_check=True)
                        t = wk.tile([P, do], f32, tag="t", name="t_t")
                        if "epi" not in skips:
                            nc.vector.tensor_scalar(
                            out=t[:], in0=out2[:],
                            scalar1=scol_t[:, scol_off + jj:scol_off + jj + 1],
                            scalar2=None, op0=mybir.AluOpType.mult)
                        u = wk.tile([P, do], f32, tag="u", name="u_t")
                        nc.scalar.activation(
                            u[:], out2[:], mybir.ActivationFunctionType.Copy,
                            scale=scol_t[:, scol_off + NCH + jj:scol_off + NCH + jj + 1])
                        if l < NL:
                            hp = wk.tile([P, do], bf16, tag="hp", name="hp")
                            nc.vector.tensor_tensor(out=hp[:], in0=t[:], in1=u[:],
                                                    op=mybir.AluOpType.max)
                            r0 = jj * P
                            r1 = min(r0 + P, LOC)
                            if r1 > r0:
                                nc.sync.dma_start(agi[l - 1][r0:r1, 0:do],
                                                  hp[0:r1 - r0, :])
                        else:
                            hp32 = wk.tile([P, do], f32, tag="hp32", name="hp32")
                            nc.vector.tensor_tensor(out=hp32[:], in0=t[:], in1=u[:],
                                                    op=mybir.AluOpType.max)
                            nc.tensor.matmul(pool_ps[:],
                                             lhsT=gw_t[:, jj * 64:(jj + 1) * 64],
                                             rhs=hp32[:], start=jj == 0,
                                             stop=jj == NCH - 1,
                                             skip_group_check=True)
                if l < NL:
                    if os.environ.get("GCN_NO_CC") != "1":
                        nc.gpsimd.collective_compute(
                            "AllGather", mybir.AluOpType.bypass,
                            replica_groups=[list(range(NCORES))],
                            ins=[agi[l - 1][:]], outs=[ago[l - 1][:]])
                    prev = ago[l - 1]
            pool_sb = wk.tile([64, d_last], f32, tag="poolsb", name="pool_sb")
            nc.scalar.activation(pool_sb[:], pool_ps[:],
                                 mybir.ActivationFunctionType.Copy)
            nc.sync.dma_start(out_pool[:], pool_sb[:])
    nc.compile()
    return nc


def host_prep(inputs, src, dst, graph_ids, dims, N, E, B):
    """Build plan + per-core input maps."""
    src = np.asarray(src).astype(np.int64)
    dst = np.asarray(dst).astype(np.int64)
    gid = np.asarray(graph_ids).astype(np.int64)
    plan = Plan(N, E, B, dims, src, dst, gid)
    out_deg = np.maximum(np.bincount(src, minlength=N), 1).astype(np.float32)
    in_deg = np.maximum(np.bincount(dst, minlength=N), 1).astype(np.float32)
    o = out_deg ** -0.5
    i = in_deg ** -0.5
    s10 = (i * o).astype(np.float32)
    s11 = i.astype(np.float32)
    LOC, NCH = plan.LOC, plan.NCH
    scol = np.zeros((NCORES, P, NCH * 4), np.float32)
    invi = np.zeros((NCORES, 1, NCH * P), BF)
    gw = np.zeros((NCORES, P, NCH * 64), np.float32)
    for c in range(NCORES):
        loc_n = np.arange(LOC) + c * LOC
        for j in range(NCH):
            r0, r1 = j * P, min((j + 1) * P, LOC)
            npart = r1 - r0
            scol[c, 0:npart, j] = s10[loc_n[r0:r1]]
            scol[c, 0:npart, NCH + j] = SLOPE * s10[loc_n[r0:r1]]
            scol[c, 0:npart, 2 * NCH + j] = s11[loc_n[r0:r1]]
            scol[c, 0:npart, 3 * NCH + j] = SLOPE * s11[loc_n[r0:r1]]
            invi[c, 0, j * P:j * P + npart] = (1.0 / i[loc_n[r0:r1]]).astype(BF)
            g = gid[loc_n[r0:r1]]
            gw[c, np.arange(npart), j * 64 + g] = 1.0
    iota = np.tile(np.arange(plan.win, dtype=np.float32), (P, 1)).astype(BF)
    h0 = np.zeros((N, _pd(dims[0])), BF)
    h0[:, 0:dims[0]] = (np.asarray(inputs, np.float32) * o[:, None]).astype(BF)
    return plan, dict(scol=scol, invi=invi, gw=gw, iota=iota, h0=h0)


def kernel(inputs, edata, src, dst, graph_ids, conv_weights, conv_biases,
           w1, b1, w2, b2, param_mu, param_sigma):
    import jax
    N, E, B = N_FULL, E_FULL, B_FULL
    dims = DIMS_FULL
    inputs = np.asarray(inputs, np.float32)
    plan, aux = host_prep(inputs, src, dst, graph_ids, dims, N, E, B)
    nc = build_kernel(plan)
    in_maps = []
    wb = {}
    for l in range(1, len(dims)):
        wb[f"w{l}"] = np.asarray(conv_weights[l - 1], np.float32).astype(BF)
        wb[f"b{l}"] = np.asarray(conv_biases[l - 1], np.float32).reshape(1, -1).astype(BF)
    for c in range(NCORES):
        m = dict(h0=aux["h0"], iota=aux["iota"],
                 idx16=plan.idx16[c], dstrel=plan.dstrel[c],
                 scol=aux["scol"][c], invi=aux["invi"][c], gw=aux["gw"][c])
        m.update(wb)
        in_maps.append(m)
    trace = os.environ.get("GCN_TRACE") == "1"
    try:
        res = bass_utils.run_bass_kernel_spmd(nc, in_maps,
                                              core_ids=list(range(NCORES)),
                                              trace=trace)
    except ModuleNotFoundError:
        # NTFF profiling hook unavailable in this environment
        res = bass_utils.run_bass_kernel_spmd(nc, in_maps,
                                              core_ids=list(range(NCORES)))
    global last_exec_time_ns, last_nc, last_in_maps
    last_exec_time_ns = res.exec_time_ns
    last_nc, last_in_maps = nc, in_maps
    pool = np.zeros((64, dims[-1]), np.float64)
    for c in range(NCORES):
        pool += res.results[c]["pool"].astype(np.float64)
    gid = np.asarray(graph_ids).astype(np.int64)
    counts = np.maximum(np.bincount(gid, minlength=B), 1).astype(np.float32)
    hg = (pool.astype(np.float32)) / counts[:, None]
    hg = _leaky(hg)
    hg = hg @ np.asarray(w1, np.float32) + np.asarray(b1, np.float32)
    cpu = jax.local_devices(backend="cpu")[0]
    with jax.default_device(cpu):
        keep1 = np.asarray(jax.random.bernoulli(jax.random.key(101), 0.7, hg.shape))
        keep2 = np.asarray(jax.random.bernoulli(jax.random.key(102), 0.8,
                                                (hg.shape[0], np.asarray(w2).shape[1])))
    hg = np.where(keep1, hg / 0.7, 0.0).astype(np.float32)
    hg = _leaky(hg)
    hg = hg @ np.asarray(w2, np.float32) + np.asarray(b2, np.float32)
    hg = np.where(keep2, hg / 0.8, 0.0).astype(np.float32)
    out = 1.0 / (1.0 + np.exp(-hg))
    return out.astype(np.float32)
